# revision 20
# baseline (speedup 1.0000x reference)
"""Trainium2 Bass kernel for nn_DeepIM (VAE + 2-layer rank-1 GAT on 4096 nodes).

Strategy (8 NeuronCores, SPMD):
  - VAE: mat-vec layers row-sharded 8 ways (memory-bound weight streaming
    split across cores); each layer output shard is AllGathered (~KB) and
    broadcast across partitions for the next layer.
  - GAT: Wh = x_hat[:,None] @ W is rank-1, so e_ij = c1*x_i + c2*x_j with
    host-precomputed scalars from the tiny W/a params. Each core owns
    N/8 = 512 query rows. Per (head, row-tile): the adjacency mask enters
    additively (adj ? 0 : -1e30) before Lrelu, ScalarE computes
    Lrelu -> Exp with a free row-sum (accum_out) giving the softmax
    denominator; the numerator uses exp(l + ln x_j) with one more fused
    Exp+accum. Softmax max-subtraction cancels in numer/denom, so skipped.
  - Layer 2 collapses identically: Wh2 = sum_h d_h * t_h (can be negative,
    so ln uses a +S shift, subtracted back out of the accumulated sum).

Compiler workarounds for this walrus build:
  - every instruction may carry at most ONE sync-wait: extra waits are
    split onto prefix NoOps on the same engine (see _split_waits_list).
  - tensor_tensor_reduce / raw-ISA DVE ops are broken ("ISA wrong length"):
    reductions use activation accum_out instead.
"""

import sys

if "/opt/trn_rl_repo" not in sys.path:
    sys.path.insert(0, "/opt/trn_rl_repo")

import numpy as np

import concourse.bass as bass
import concourse.mybir as mybir
import concourse.tile as tile
from concourse.vector_clock import ScopedClock
from concourse.bass_utils import run_bass_kernel_spmd

N = 4096
HID = 1024
LAT = 512
NH = 4
ALPHA = 0.2
NC = 8
R = N // NC          # 512 rows per core
RT = R // 128        # 4 row tiles per core
F32 = mybir.dt.float32
BF16 = mybir.dt.bfloat16
U8 = mybir.dt.uint8
NEG = -100.0

# ---------------------------------------------------------------------------
# Tile patches for this walrus build (1 sync-wait per instruction max).
# ---------------------------------------------------------------------------


def _split_waits_list(nc, insts):
    out = []
    for inst in insts:
        si = getattr(inst, "sync_info", None)
        if (
            si is not None
            and si.on_wait is not None
            and len(si.on_wait) > 1
            and getattr(inst, "engine", None) is not None
        ):
            waits = list(si.on_wait)
            for w in waits[:-1]:
                nop = mybir.InstNoOp(
                    name=nc.get_next_instruction_name(),
                    sync_info=mybir.SyncInfo(on_wait=[w], on_update=[]),
                    bass_nofuse=True,
                    engine=inst.engine,
                )
                out.append(nop)
            inst.sync_info = mybir.SyncInfo(
                on_wait=[waits[-1]], on_update=list(si.on_update or [])
            )
        out.append(inst)
    return out


_orig_lower = tile.TileContext._lower_ordered_insts


def _patched_lower(self, ordered):
    for bb in list(ordered.keys()):
        ordered[bb] = _split_waits_list(self.nc, ordered[bb])
    return _orig_lower(self, ordered)


def _drain_and_barrier_split(self, tick_clock, wait_clock):
    drain_inst = self.nc.sync.drain()
    wait_clock.add_sem_waits(
        drain_inst.ins, ScopedClock({None: tick_clock.global_clock})
    )
    si = drain_inst.ins.sync_info
    if si is not None and si.on_wait is not None and len(si.on_wait) > 1:
        waits = list(si.on_wait)
        drain_inst.ins.sync_info = mybir.SyncInfo(
            on_wait=waits[:1], on_update=list(si.on_update or [])
        )
        for i in range(1, len(waits)):
            d2 = self.nc.sync.drain()
            d2.ins.sync_info = mybir.SyncInfo(on_wait=[waits[i]], on_update=[])
    self.nc.all_engine_barrier()
    assert self.sems is not None
    popped = self.nc._tile_sem_poison_stack.pop()
    assert popped is self._sem_poison
    self.nc.clear_and_free_semaphores(list(self.sems.allocated().values()))
    self.nc.all_engine_barrier()


tile.TileContext._lower_ordered_insts = _patched_lower
tile.TileContext._drain_and_barrier = _drain_and_barrier_split


def _bcast_ap(ap, count, parts=128):
    """`count` contiguous elements at `ap`, replicated across partitions."""
    return bass.AP(tensor=ap.tensor, offset=ap.offset, ap=[[0, parts], [1, count]])


def _col_ap(ap, n):
    """[n] contiguous DRAM vector viewed as an [n, 1] column AP."""
    return bass.AP(tensor=ap.tensor, offset=ap.offset, ap=[[1, n], [1, 1]])


def _build_program(c1, c2, dd, a0, a1, s_shift):
    nc = bass.Bass(trn_type="TRN2", num_devices=NC)

    i_w1 = nc.dram_tensor("w1", [128, N], F32, kind="ExternalInput")
    i_b1 = nc.dram_tensor("b1", [128], F32, kind="ExternalInput")
    i_w2 = nc.dram_tensor("w2", [HID, HID], F32, kind="ExternalInput")
    i_b2 = nc.dram_tensor("b2", [HID], F32, kind="ExternalInput")
    i_w34 = nc.dram_tensor("w34", [HID, HID], F32, kind="ExternalInput")
    i_b34 = nc.dram_tensor("b34", [HID], F32, kind="ExternalInput")
    i_dw1 = nc.dram_tensor("dw1", [HID, LAT], F32, kind="ExternalInput")
    i_db1 = nc.dram_tensor("db1", [HID], F32, kind="ExternalInput")
    i_dw2 = nc.dram_tensor("dw2", [HID, HID], F32, kind="ExternalInput")
    i_db2 = nc.dram_tensor("db2", [HID], F32, kind="ExternalInput")
    i_dw3 = nc.dram_tensor("dw3", [R, HID], F32, kind="ExternalInput")
    i_db3 = nc.dram_tensor("db3", [R], F32, kind="ExternalInput")
    i_x = nc.dram_tensor("x", [N], F32, kind="ExternalInput")
    i_eps = nc.dram_tensor("eps", [LAT], F32, kind="ExternalInput")
    i_adj = nc.dram_tensor("adj", [R, N], U8, kind="ExternalInput")

    o_xhat = nc.dram_tensor("o_xhat", [R], F32, kind="ExternalOutput")
    o_mulv = nc.dram_tensor("o_mulv", [HID], F32, kind="ExternalOutput")
    o_y = nc.dram_tensor("o_y", [R], F32, kind="ExternalOutput")

    AF = mybir.ActivationFunctionType
    OP = mybir.AluOpType
    rg = [list(range(NC))]

    with tile.TileContext(nc) as tc:
        with (
            tc.tile_pool(name="pers", bufs=1) as pers,
            tc.tile_pool(name="cols", bufs=1) as cols,
            tc.tile_pool(name="dram", bufs=1, space="DRAM") as dram,
        ):
            madj_t = []
            xh_cols = []
            xh_out = dram.tile([N], F32, tag="co_xh")
            with (
                tc.tile_pool(name="wpool", bufs=2) as wpool,
                tc.tile_pool(name="wpool1", bufs=1) as wpool1,
                tc.tile_pool(name="bc", bufs=1) as bc,
            ):
                # ---- additive adjacency mask: adj ? 0 : -1e30 (bf16) ----
                # (independent of the VAE chain; the scheduler runs it early)
                for t in range(RT):
                    a8 = wpool1.tile([128, N], U8, tag="a8")
                    nc.gpsimd.dma_start(a8[:], i_adj[t * 128 : (t + 1) * 128, :])
                    af = wpool1.tile([128, N], F32, tag="af")
                    nc.vector.tensor_copy(af[:], a8[:])
                    mt = pers.tile([128, N], BF16, tag=f"madj{t}")
                    nc.vector.tensor_scalar(mt[:], af[:], -NEG, NEG, OP.mult, OP.add)
                    madj_t.append(mt)

                # ---------- VAE ----------
                def vae_layer(w_ap, b_ap, in_b, relu, tag, kcols, wtag="w", wp=None):
                    """mat-vec: h[p] = relu(sum_k w[p,k]*in_b[p,k] + b[p])."""
                    wt = (wp or wpool).tile([128, kcols], F32, tag=wtag)
                    nc.gpsimd.dma_start(wt[:], w_ap)
                    bcol = cols.tile([128, 1], F32, tag=f"b_{tag}")
                    nc.gpsimd.dma_start(bcol[:], _col_ap(b_ap, 128))
                    q = wpool1.tile([128, kcols], F32, tag="af")
                    nc.vector.tensor_tensor(q[:], wt[:], in_b[:, :kcols], OP.mult)
                    hcol = cols.tile([128, 1], F32, tag=f"h_{tag}")
                    nc.scalar.activation(q[:], q[:], AF.Copy, accum_out=hcol[:])
                    nc.vector.tensor_tensor(hcol[:], hcol[:], bcol[:], OP.add)
                    if relu:
                        nc.vector.tensor_scalar_max(hcol[:], hcol[:], 0.0)
                    return hcol

                def ag_bcast(hcol, shard, tag):
                    full = shard * NC
                    cin = dram.tile([shard], F32, tag=f"ci_{tag}")
                    cout = dram.tile([full], F32, tag=f"co_{tag}")
                    nc.gpsimd.dma_start(_col_ap(cin[:], shard), hcol[:shard, :])
                    nc.gpsimd.collective_compute(
                        "AllGather", OP.bypass, replica_groups=rg,
                        ins=[cin[:].opt()], outs=[cout[:].opt()],
                    )
                    hb = bc.tile([128, full], F32, tag="hb")
                    nc.gpsimd.dma_start(hb[:], _bcast_ap(cout[:], full))
                    return hb, cout

                def vae_layer_repl(w_h, b_h, in_b, relu, tag, kcols, nout):
                    """replicated mat-vec: every core computes all `nout`
                    outputs; result returned as a [128, nout] broadcast tile
                    via a local DRAM bounce (no collective)."""
                    nj = nout // 128
                    bounce = dram.tile([nout], F32, tag=f"bn_{tag}")
                    for j in range(nj):
                        hc = vae_layer(
                            w_h[j * 128 : (j + 1) * 128, :],
                            b_h[j * 128 : (j + 1) * 128],
                            in_b, relu, f"{tag}{j}", kcols,
                        )
                        nc.gpsimd.dma_start(
                            _col_ap(bounce[j * 128 : (j + 1) * 128], 128), hc[:]
                        )
                    hb = bc.tile([128, nout], F32, tag="hb")
                    nc.gpsimd.dma_start(hb[:], _bcast_ap(bounce[:], nout))
                    return hb, bounce

                x_b = wpool1.tile([128, N], F32, tag="xbv")
                nc.gpsimd.dma_start(x_b[:], _bcast_ap(i_x[:], N))

                h1 = vae_layer(i_w1[:, :], i_b1[:], x_b, True, "h1", N, wtag="w1", wp=wpool1)
                h1b, _ = ag_bcast(h1, 128, "h1")
                h2b, _ = vae_layer_repl(i_w2, i_b1 if False else i_b2, h1b, True, "h2", HID, HID)
                mlb, mlbounce = vae_layer_repl(i_w34, i_b34, h2b, False, "ml", HID, HID)
                nc.gpsimd.dma_start(o_mulv[:], mlbounce[:])

                # z = mu + eps * exp(0.5*logvar)
                eps_b = bc.tile([128, LAT], F32, tag="epsb")
                nc.gpsimd.dma_start(eps_b[:], _bcast_ap(i_eps[:], LAT))
                zb = bc.tile([128, LAT], F32, tag="zb")
                nc.scalar.activation(zb[:], mlb[:, LAT:], AF.Exp, scale=0.5)
                nc.vector.tensor_tensor(zb[:], zb[:], eps_b[:], OP.mult)
                nc.vector.tensor_tensor(zb[:], zb[:], mlb[:, :LAT], OP.add)

                h3b, _ = vae_layer_repl(i_dw1, i_db1, zb, True, "h3", LAT, HID)
                h4b, _ = vae_layer_repl(i_dw2, i_db2, h3b, True, "h4", HID, HID)

                for t in range(RT):
                    hc = vae_layer(
                        i_dw3[t * 128 : (t + 1) * 128, :],
                        i_db3[t * 128 : (t + 1) * 128],
                        h4b, False, f"d3_{t}", HID,
                    )
                    # sigmoid(v) = 1/(1+exp(-v))
                    sg = cols.tile([128, 1], F32, tag=f"sg_{t}")
                    nc.scalar.activation(sg[:], hc[:], AF.Exp, scale=-1.0)
                    nc.vector.tensor_scalar_add(sg[:], sg[:], 1.0)
                    nc.vector.reciprocal(sg[:], sg[:])
                    nc.gpsimd.dma_start(_col_ap(o_xhat[t * 128 : (t + 1) * 128], 128), sg[:])
                    xh_cols.append(sg)

                xh_in = dram.tile([R], F32, tag="ci_xh")
                for t in range(RT):
                    nc.gpsimd.dma_start(
                        _col_ap(xh_in[t * 128 : (t + 1) * 128], 128), xh_cols[t][:]
                    )
                nc.gpsimd.collective_compute(
                    "AllGather", OP.bypass, replica_groups=rg,
                    ins=[xh_in[:].opt()], outs=[xh_out[:].opt()],
                )

            # ---------- GAT ----------
            with tc.tile_pool(name="attp", bufs=1) as attp:
                # ln(x_hat) broadcast (numerator trick)
                stg = attp.tile([128, N], F32, tag="l2", bufs=2)
                nc.gpsimd.dma_start(stg[:], _bcast_ap(xh_out[:], N))
                lnx_b = pers.tile([128, N], F32, tag="lnx")
                nc.scalar.activation(lnx_b[:], stg[:], AF.Ln)

                def att_pass(src_dram, lnvb, vcols, s1_scale, s2_scale, tag):
                    """per row tile: l = prelu(s2*v_j + s1*v_i + madj, 0.2);
                    den = sum_j exp(l); num = sum_j exp(l + lnv_j)."""
                    xcf = attp.tile([128, N], F32, tag="xcf", bufs=2)
                    nc.gpsimd.dma_start(xcf[:], _bcast_ap(src_dram[:], N))
                    xc2 = attp.tile([128, N], BF16, tag="xc2", bufs=2)
                    nc.vector.tensor_scalar_mul(xc2[:], xcf[:], s2_scale)
                    outs = []
                    for t in range(RT):
                        s1c = cols.tile([128, 1], F32, tag=f"s1_{tag}_{t}")
                        nc.vector.tensor_scalar_mul(s1c[:], vcols[t][:], s1_scale)
                        tm = attp.tile([128, N], BF16, tag="tm", bufs=2)
                        nc.vector.tensor_tensor(tm[:], xc2[:], madj_t[t][:], OP.add)
                        lt = attp.tile([128, N], F32, tag="lt", bufs=1)
                        nc.scalar.activation(
                            lt[:], tm[:], AF.Prelu, bias=s1c[:], alpha=ALPHA
                        )
                        l2 = attp.tile([128, N], F32, tag="l2", bufs=2)
                        nc.vector.tensor_tensor(l2[:], lt[:], lnvb[:], OP.add)
                        den = cols.tile([128, 1], F32, tag=f"den_{tag}_{t}")
                        gs = attp.tile([128, N], BF16, tag="gs", bufs=1)
                        nc.scalar.activation(gs[:], lt[:], AF.Exp, accum_out=den[:])
                        num = cols.tile([128, 1], F32, tag=f"num_{tag}_{t}")
                        nc.scalar.activation(l2[:], l2[:], AF.Exp, accum_out=num[:])
                        outs.append((num, den))
                    return outs

                # layer 1: Wh2 = sum_h d_h * (num/den)
                wh2_cols = []
                for t in range(RT):
                    wc = cols.tile([128, 1], F32, tag=f"wh2_{t}")
                    nc.vector.memset(wc[:], 0.0)
                    wh2_cols.append(wc)
                for h in range(NH):
                    nd = att_pass(xh_out, lnx_b, xh_cols, c1[h], c2[h], f"h{h}")
                    for t, (num, den) in enumerate(nd):
                        r = cols.tile([128, 1], F32, tag=f"r_{t}")
                        nc.vector.reciprocal(r[:], den[:])
                        nc.vector.tensor_tensor(r[:], r[:], num[:], OP.mult)
                        nc.vector.tensor_scalar_mul(r[:], r[:], float(dd[h]))
                        nc.vector.tensor_tensor(wh2_cols[t][:], wh2_cols[t][:], r[:], OP.add)

                # gather Wh2
                w_in = dram.tile([R], F32, tag="ci_w2")
                w_out = dram.tile([N], F32, tag="co_w2")
                for t in range(RT):
                    nc.gpsimd.dma_start(
                        _col_ap(w_in[t * 128 : (t + 1) * 128], 128), wh2_cols[t][:]
                    )
                nc.gpsimd.collective_compute(
                    "AllGather", OP.bypass, replica_groups=rg,
                    ins=[w_in[:].opt()], outs=[w_out[:].opt()],
                )
                # ln(Wh2 + S) for the numerator (Wh2 can be negative)
                wsb = attp.tile([128, N], F32, tag="l2", bufs=2)
                nc.gpsimd.dma_start(wsb[:], _bcast_ap(w_out[:], N))
                nc.vector.tensor_scalar_add(wsb[:], wsb[:], float(s_shift))
                lnw_b = pers.tile([128, N], F32, tag="lnx")  # reuse lnx slot
                nc.scalar.activation(lnw_b[:], wsb[:], AF.Ln)

                nd2 = att_pass(w_out, lnw_b, wh2_cols, float(a0), float(a1), "l2")
                for t, (num, den) in enumerate(nd2):
                    # true num = num_accum - S*den ; q = num/den
                    sden = cols.tile([128, 1], F32, tag=f"sd_{t}")
                    nc.vector.tensor_scalar_mul(sden[:], den[:], float(s_shift))
                    q = cols.tile([128, 1], F32, tag=f"q_{t}")
                    nc.vector.tensor_tensor(q[:], num[:], sden[:], OP.subtract)
                    r2 = cols.tile([128, 1], F32, tag=f"r2_{t}")
                    nc.vector.reciprocal(r2[:], den[:])
                    nc.vector.tensor_tensor(q[:], q[:], r2[:], OP.mult)
                    # elu(q) = max(q,0) + (exp(min(q,0)) - 1)
                    qn = cols.tile([128, 1], F32, tag=f"qn_{t}")
                    nc.vector.tensor_scalar_min(qn[:], q[:], 0.0)
                    nc.scalar.activation(qn[:], qn[:], AF.Exp)
                    nc.vector.tensor_scalar_add(qn[:], qn[:], -1.0)
                    nc.vector.tensor_scalar_max(q[:], q[:], 0.0)
                    nc.vector.tensor_tensor(q[:], q[:], qn[:], OP.add)
                    nc.gpsimd.dma_start(_col_ap(o_y[t * 128 : (t + 1) * 128], 128), q[:])

    return nc


def _ensure_ntff_hook():
    """Make trace=True work under axon: provide antenv.axon_hooks and
    register the ctypes NTFF hook from the boot helper (test-only path)."""
    import types, importlib
    try:
        from antenv.axon_hooks import get_axon_ntff_profile_hook  # noqa
        return
    except ImportError:
        pass
    import antenv
    mod = types.ModuleType("antenv.axon_hooks")
    mod._hook = None

    def set_axon_ntff_profile_hook(hook):
        mod._hook = hook

    def get_axon_ntff_profile_hook():
        return mod._hook

    mod.set_axon_ntff_profile_hook = set_axon_ntff_profile_hook
    mod.get_axon_ntff_profile_hook = get_axon_ntff_profile_hook
    sys.modules["antenv.axon_hooks"] = mod
    antenv.axon_hooks = mod
    try:
        if "/root/.axon_site" not in sys.path:
            sys.path.insert(0, "/root/.axon_site")
        from trn_agent_boot.trn_boot import _ntff_profile_via_ctypes
        hook = _ntff_profile_via_ctypes("/opt/axon/libaxon_pjrt.so")
        if hook is not None:
            set_axon_ntff_profile_hook(hook)
    except Exception as e:  # degrade: trace skipped, run still works
        print("ntff hook setup failed:", e)


def kernel(x, eps, adj, enc_w1, enc_b1, enc_w2, enc_b2, enc_w3, enc_b3,
           enc_w4, enc_b4, dec_w1, dec_b1, dec_w2, dec_b2, dec_w3, dec_b3,
           gat_W, gat_a, out_W, out_a, _trace=False):
    x = np.ascontiguousarray(np.asarray(x, np.float32))
    eps = np.ascontiguousarray(np.asarray(eps, np.float32))
    adj_u8 = np.ascontiguousarray(np.asarray(adj).astype(np.uint8))
    gat_W = np.asarray(gat_W, np.float32)
    gat_a = np.asarray(gat_a, np.float32)
    out_W = np.asarray(out_W, np.float32)
    out_a = np.asarray(out_a, np.float32)

    c1 = [float(gat_W[h, 0, :] @ gat_a[h, :64, 0]) for h in range(NH)]
    c2 = [float(gat_W[h, 0, :] @ gat_a[h, 64:, 0]) for h in range(NH)]
    dd = [float(gat_W[h, 0, :] @ out_W[h * 64:(h + 1) * 64, 0]) for h in range(NH)]
    a0 = float(out_a[0, 0])
    a1 = float(out_a[1, 0])
    # |Wh2| = |sum_h d_h t_h| < sum |d_h| since t_h is a convex combination
    # of x_hat values in (0,1); pad a little so ln(Wh2+S) stays finite.
    s_shift = float(sum(abs(d) for d in dd) + 1.0)

    w34 = np.concatenate([np.asarray(enc_w3), np.asarray(enc_w4)], 0)
    b34 = np.concatenate([np.asarray(enc_b3), np.asarray(enc_b4)], 0)

    nc = _build_program(c1, c2, dd, a0, a1, s_shift)

    def shard(arr, c, rows):
        return np.ascontiguousarray(np.asarray(arr, np.float32)[c * rows:(c + 1) * rows])

    in_maps = []
    for c in range(NC):
        in_maps.append({
            "w1": shard(enc_w1, c, 128), "b1": shard(enc_b1, c, 128),
            "w2": np.ascontiguousarray(np.asarray(enc_w2, np.float32)),
            "b2": np.ascontiguousarray(np.asarray(enc_b2, np.float32)),
            "w34": np.ascontiguousarray(w34.astype(np.float32)),
            "b34": np.ascontiguousarray(b34.astype(np.float32)),
            "dw1": np.ascontiguousarray(np.asarray(dec_w1, np.float32)),
            "db1": np.ascontiguousarray(np.asarray(dec_b1, np.float32)),
            "dw2": np.ascontiguousarray(np.asarray(dec_w2, np.float32)),
            "db2": np.ascontiguousarray(np.asarray(dec_b2, np.float32)),
            "dw3": shard(dec_w3, c, R), "db3": shard(dec_b3, c, R),
            "x": x, "eps": eps,
            "adj": np.ascontiguousarray(adj_u8[c * R:(c + 1) * R]),
        })

    if _trace is True:
        _ensure_ntff_hook()
    res = run_bass_kernel_spmd(nc, in_maps, core_ids=list(range(NC)),
                               trace=(_trace is True))

    xhat = np.concatenate([res.results[c]["o_xhat"] for c in range(NC)])
    mulv = res.results[0]["o_mulv"]
    y = np.concatenate([res.results[c]["o_y"] for c in range(NC)])
    out = (xhat.astype(np.float32),
           y.astype(np.float32)[:, None],
           mulv[:LAT].astype(np.float32),
           mulv[LAT:].astype(np.float32))
    if _trace:
        return out, res
    return out


# revision 21
# speedup vs baseline: 1.0981x; 1.0981x over previous
"""Trainium2 Bass kernel for nn_DeepIM (VAE + 2-layer rank-1 GAT on 4096 nodes).

Strategy (8 NeuronCores, SPMD):
  - VAE: mat-vec layers row-sharded 8 ways (memory-bound weight streaming
    split across cores); each layer output shard is AllGathered (~KB) and
    broadcast across partitions for the next layer.
  - GAT: Wh = x_hat[:,None] @ W is rank-1, so e_ij = c1*x_i + c2*x_j with
    host-precomputed scalars from the tiny W/a params. Each core owns
    N/8 = 512 query rows. Per (head, row-tile): the adjacency mask enters
    additively (adj ? 0 : -1e30) before Lrelu, ScalarE computes
    Lrelu -> Exp with a free row-sum (accum_out) giving the softmax
    denominator; the numerator uses exp(l + ln x_j) with one more fused
    Exp+accum. Softmax max-subtraction cancels in numer/denom, so skipped.
  - Layer 2 collapses identically: Wh2 = sum_h d_h * t_h (can be negative,
    so ln uses a +S shift, subtracted back out of the accumulated sum).

Compiler workarounds for this walrus build:
  - every instruction may carry at most ONE sync-wait: extra waits are
    split onto prefix NoOps on the same engine (see _split_waits_list).
  - tensor_tensor_reduce / raw-ISA DVE ops are broken ("ISA wrong length"):
    reductions use activation accum_out instead.
"""

import sys

if "/opt/trn_rl_repo" not in sys.path:
    sys.path.insert(0, "/opt/trn_rl_repo")

import numpy as np

import concourse.bass as bass
import concourse.mybir as mybir
import concourse.tile as tile
from concourse.vector_clock import ScopedClock
from concourse.bass_utils import run_bass_kernel_spmd

N = 4096
HID = 1024
LAT = 512
NH = 4
ALPHA = 0.2
NC = 8
R = N // NC          # 512 rows per core
RT = R // 128        # 4 row tiles per core
F32 = mybir.dt.float32
BF16 = mybir.dt.bfloat16
U8 = mybir.dt.uint8
NEG = -100.0

# ---------------------------------------------------------------------------
# Tile patches for this walrus build (1 sync-wait per instruction max).
# ---------------------------------------------------------------------------


def _split_waits_list(nc, insts):
    out = []
    for inst in insts:
        si = getattr(inst, "sync_info", None)
        if (
            si is not None
            and si.on_wait is not None
            and len(si.on_wait) > 1
            and getattr(inst, "engine", None) is not None
        ):
            waits = list(si.on_wait)
            for w in waits[:-1]:
                nop = mybir.InstNoOp(
                    name=nc.get_next_instruction_name(),
                    sync_info=mybir.SyncInfo(on_wait=[w], on_update=[]),
                    bass_nofuse=True,
                    engine=inst.engine,
                )
                out.append(nop)
            inst.sync_info = mybir.SyncInfo(
                on_wait=[waits[-1]], on_update=list(si.on_update or [])
            )
        out.append(inst)
    return out


_orig_lower = tile.TileContext._lower_ordered_insts


def _patched_lower(self, ordered):
    for bb in list(ordered.keys()):
        ordered[bb] = _split_waits_list(self.nc, ordered[bb])
    return _orig_lower(self, ordered)


def _drain_and_barrier_split(self, tick_clock, wait_clock):
    drain_inst = self.nc.sync.drain()
    wait_clock.add_sem_waits(
        drain_inst.ins, ScopedClock({None: tick_clock.global_clock})
    )
    si = drain_inst.ins.sync_info
    if si is not None and si.on_wait is not None and len(si.on_wait) > 1:
        waits = list(si.on_wait)
        drain_inst.ins.sync_info = mybir.SyncInfo(
            on_wait=waits[:1], on_update=list(si.on_update or [])
        )
        for i in range(1, len(waits)):
            d2 = self.nc.sync.drain()
            d2.ins.sync_info = mybir.SyncInfo(on_wait=[waits[i]], on_update=[])
    self.nc.all_engine_barrier()
    assert self.sems is not None
    popped = self.nc._tile_sem_poison_stack.pop()
    assert popped is self._sem_poison
    self.nc.clear_and_free_semaphores(list(self.sems.allocated().values()))
    self.nc.all_engine_barrier()


tile.TileContext._lower_ordered_insts = _patched_lower
tile.TileContext._drain_and_barrier = _drain_and_barrier_split


def _bcast_ap(ap, count, parts=128):
    """`count` contiguous elements at `ap`, replicated across partitions."""
    return bass.AP(tensor=ap.tensor, offset=ap.offset, ap=[[0, parts], [1, count]])


def _col_ap(ap, n):
    """[n] contiguous DRAM vector viewed as an [n, 1] column AP."""
    return bass.AP(tensor=ap.tensor, offset=ap.offset, ap=[[1, n], [1, 1]])


def _build_program(c1, c2, dd, a0, a1, s_shift):
    nc = bass.Bass(trn_type="TRN2", num_devices=NC)

    i_w1 = nc.dram_tensor("w1", [128, N], F32, kind="ExternalInput")
    i_b1 = nc.dram_tensor("b1", [128], F32, kind="ExternalInput")
    i_w2 = nc.dram_tensor("w2", [HID, HID], F32, kind="ExternalInput")
    i_b2 = nc.dram_tensor("b2", [HID], F32, kind="ExternalInput")
    i_w34 = nc.dram_tensor("w34", [HID, HID], F32, kind="ExternalInput")
    i_b34 = nc.dram_tensor("b34", [HID], F32, kind="ExternalInput")
    i_dw1 = nc.dram_tensor("dw1", [HID, LAT], F32, kind="ExternalInput")
    i_db1 = nc.dram_tensor("db1", [HID], F32, kind="ExternalInput")
    i_dw2 = nc.dram_tensor("dw2", [HID, HID], F32, kind="ExternalInput")
    i_db2 = nc.dram_tensor("db2", [HID], F32, kind="ExternalInput")
    i_dw3 = nc.dram_tensor("dw3", [R, HID], F32, kind="ExternalInput")
    i_db3 = nc.dram_tensor("db3", [R], F32, kind="ExternalInput")
    i_x = nc.dram_tensor("x", [N], F32, kind="ExternalInput")
    i_eps = nc.dram_tensor("eps", [LAT], F32, kind="ExternalInput")
    i_adj = nc.dram_tensor("adj", [R, N], U8, kind="ExternalInput")

    o_xhat = nc.dram_tensor("o_xhat", [R], F32, kind="ExternalOutput")
    o_mulv = nc.dram_tensor("o_mulv", [HID], F32, kind="ExternalOutput")
    o_y = nc.dram_tensor("o_y", [R], F32, kind="ExternalOutput")

    AF = mybir.ActivationFunctionType
    OP = mybir.AluOpType
    rg = [list(range(NC))]

    with tile.TileContext(nc) as tc:
        with (
            tc.tile_pool(name="pers", bufs=1) as pers,
            tc.tile_pool(name="cols", bufs=1) as cols,
            tc.tile_pool(name="dram", bufs=1, space="DRAM") as dram,
        ):
            madj_t = []
            xh_cols = []
            xh_out = dram.tile([N], F32, tag="co_xh")
            with (
                tc.tile_pool(name="wpool", bufs=2) as wpool,
                tc.tile_pool(name="wpool1", bufs=1) as wpool1,
                tc.tile_pool(name="bc", bufs=1) as bc,
            ):
                # ---- additive adjacency mask: adj ? 0 : -1e30 (bf16) ----
                # (independent of the VAE chain; the scheduler runs it early)
                for t in range(RT):
                    a8 = wpool1.tile([128, N], U8, tag="a8")
                    nc.sync.dma_start(a8[:], i_adj[t * 128 : (t + 1) * 128, :])
                    af = wpool1.tile([128, N], F32, tag="af", bufs=4)
                    nc.vector.tensor_copy(af[:], a8[:])
                    mt = pers.tile([128, N], BF16, tag=f"madj{t}")
                    nc.vector.tensor_scalar(mt[:], af[:], -NEG, NEG, OP.mult, OP.add)
                    madj_t.append(mt)

                # ---------- VAE ----------
                def vae_layer(w_ap, b_ap, in_b, relu, tag, kcols, wtag="w", wp=None):
                    """mat-vec: h[p] = relu(sum_k w[p,k]*in_b[p,k] + b[p])."""
                    wt = (wp or wpool).tile([128, kcols], F32, tag=wtag)
                    nc.sync.dma_start(wt[:], w_ap)
                    bcol = cols.tile([128, 1], F32, tag=f"b_{tag}")
                    nc.gpsimd.dma_start(bcol[:], _col_ap(b_ap, 128))
                    q = wpool1.tile([128, kcols], F32, tag="af", bufs=4)
                    nc.vector.tensor_tensor(q[:], wt[:], in_b[:, :kcols], OP.mult)
                    hcol = cols.tile([128, 1], F32, tag=f"h_{tag}")
                    nc.scalar.activation(q[:], q[:], AF.Copy, accum_out=hcol[:])
                    nc.vector.tensor_tensor(hcol[:], hcol[:], bcol[:], OP.add)
                    if relu:
                        nc.vector.tensor_scalar_max(hcol[:], hcol[:], 0.0)
                    return hcol

                def ag_bcast(hcol, shard, tag):
                    full = shard * NC
                    cin = dram.tile([shard], F32, tag=f"ci_{tag}")
                    cout = dram.tile([full], F32, tag=f"co_{tag}")
                    nc.gpsimd.dma_start(_col_ap(cin[:], shard), hcol[:shard, :])
                    nc.gpsimd.collective_compute(
                        "AllGather", OP.bypass, replica_groups=rg,
                        ins=[cin[:].opt()], outs=[cout[:].opt()],
                    )
                    hb = bc.tile([128, full], F32, tag="hb")
                    nc.sync.dma_start(hb[:], _bcast_ap(cout[:], full))
                    return hb, cout

                def vae_layer_repl(w_h, b_h, in_b, relu, tag, kcols, nout):
                    """replicated mat-vec: every core computes all `nout`
                    outputs; result returned as a [128, nout] broadcast tile
                    via a local DRAM bounce (no collective)."""
                    nj = nout // 128
                    bounce = dram.tile([nout], F32, tag=f"bn_{tag}")
                    for j in range(nj):
                        hc = vae_layer(
                            w_h[j * 128 : (j + 1) * 128, :],
                            b_h[j * 128 : (j + 1) * 128],
                            in_b, relu, f"{tag}{j}", kcols,
                        )
                        nc.gpsimd.dma_start(
                            _col_ap(bounce[j * 128 : (j + 1) * 128], 128), hc[:]
                        )
                    hb = bc.tile([128, nout], F32, tag="hb")
                    nc.sync.dma_start(hb[:], _bcast_ap(bounce[:], nout))
                    return hb, bounce

                x_b = wpool1.tile([128, N], F32, tag="xbv")
                nc.sync.dma_start(x_b[:], _bcast_ap(i_x[:], N))

                h1 = vae_layer(i_w1[:, :], i_b1[:], x_b, True, "h1", N, wtag="w1", wp=wpool1)
                h1b, _ = ag_bcast(h1, 128, "h1")
                h2b, _ = vae_layer_repl(i_w2, i_b1 if False else i_b2, h1b, True, "h2", HID, HID)
                mlb, mlbounce = vae_layer_repl(i_w34, i_b34, h2b, False, "ml", HID, HID)
                nc.gpsimd.dma_start(o_mulv[:], mlbounce[:])

                # z = mu + eps * exp(0.5*logvar)
                eps_b = bc.tile([128, LAT], F32, tag="epsb")
                nc.sync.dma_start(eps_b[:], _bcast_ap(i_eps[:], LAT))
                zb = bc.tile([128, LAT], F32, tag="zb")
                nc.scalar.activation(zb[:], mlb[:, LAT:], AF.Exp, scale=0.5)
                nc.vector.tensor_tensor(zb[:], zb[:], eps_b[:], OP.mult)
                nc.vector.tensor_tensor(zb[:], zb[:], mlb[:, :LAT], OP.add)

                h3b, _ = vae_layer_repl(i_dw1, i_db1, zb, True, "h3", LAT, HID)
                h4b, _ = vae_layer_repl(i_dw2, i_db2, h3b, True, "h4", HID, HID)

                for t in range(RT):
                    hc = vae_layer(
                        i_dw3[t * 128 : (t + 1) * 128, :],
                        i_db3[t * 128 : (t + 1) * 128],
                        h4b, False, f"d3_{t}", HID,
                    )
                    # sigmoid(v) = 1/(1+exp(-v))
                    sg = cols.tile([128, 1], F32, tag=f"sg_{t}")
                    nc.scalar.activation(sg[:], hc[:], AF.Exp, scale=-1.0)
                    nc.vector.tensor_scalar_add(sg[:], sg[:], 1.0)
                    nc.vector.reciprocal(sg[:], sg[:])
                    nc.gpsimd.dma_start(_col_ap(o_xhat[t * 128 : (t + 1) * 128], 128), sg[:])
                    xh_cols.append(sg)

                xh_in = dram.tile([R], F32, tag="ci_xh")
                for t in range(RT):
                    nc.gpsimd.dma_start(
                        _col_ap(xh_in[t * 128 : (t + 1) * 128], 128), xh_cols[t][:]
                    )
                nc.gpsimd.collective_compute(
                    "AllGather", OP.bypass, replica_groups=rg,
                    ins=[xh_in[:].opt()], outs=[xh_out[:].opt()],
                )

            # ---------- GAT ----------
            with tc.tile_pool(name="attp", bufs=1) as attp:
                # ln(x_hat) broadcast (numerator trick)
                stg = attp.tile([128, N], F32, tag="l2", bufs=2)
                nc.sync.dma_start(stg[:], _bcast_ap(xh_out[:], N))
                lnx_b = pers.tile([128, N], F32, tag="lnx")
                nc.scalar.activation(lnx_b[:], stg[:], AF.Ln)

                def att_pass(src_dram, lnvb, vcols, s1_scale, s2_scale, tag):
                    """per row tile: l = prelu(s2*v_j + s1*v_i + madj, 0.2);
                    den = sum_j exp(l); num = sum_j exp(l + lnv_j)."""
                    xcf = attp.tile([128, N], F32, tag="xcf", bufs=2)
                    nc.sync.dma_start(xcf[:], _bcast_ap(src_dram[:], N))
                    xc2 = attp.tile([128, N], BF16, tag="xc2", bufs=2)
                    nc.vector.tensor_scalar_mul(xc2[:], xcf[:], s2_scale)
                    outs = []
                    for t in range(RT):
                        s1c = cols.tile([128, 1], F32, tag=f"s1_{tag}_{t}")
                        nc.vector.tensor_scalar_mul(s1c[:], vcols[t][:], s1_scale)
                        tm = attp.tile([128, N], BF16, tag="tm", bufs=2)
                        nc.vector.tensor_tensor(tm[:], xc2[:], madj_t[t][:], OP.add)
                        lt = attp.tile([128, N], F32, tag="lt", bufs=1)
                        nc.scalar.activation(
                            lt[:], tm[:], AF.Prelu, bias=s1c[:], alpha=ALPHA
                        )
                        l2 = attp.tile([128, N], F32, tag="l2", bufs=2)
                        nc.vector.tensor_tensor(l2[:], lt[:], lnvb[:], OP.add)
                        den = cols.tile([128, 1], F32, tag=f"den_{tag}_{t}")
                        gs = attp.tile([128, N], BF16, tag="gs", bufs=1)
                        nc.scalar.activation(gs[:], lt[:], AF.Exp, accum_out=den[:])
                        num = cols.tile([128, 1], F32, tag=f"num_{tag}_{t}")
                        nc.scalar.activation(l2[:], l2[:], AF.Exp, accum_out=num[:])
                        outs.append((num, den))
                    return outs

                # layer 1: Wh2 = sum_h d_h * (num/den)
                wh2_cols = []
                for t in range(RT):
                    wc = cols.tile([128, 1], F32, tag=f"wh2_{t}")
                    nc.vector.memset(wc[:], 0.0)
                    wh2_cols.append(wc)
                for h in range(NH):
                    nd = att_pass(xh_out, lnx_b, xh_cols, c1[h], c2[h], f"h{h}")
                    for t, (num, den) in enumerate(nd):
                        r = cols.tile([128, 1], F32, tag=f"r_{t}")
                        nc.vector.reciprocal(r[:], den[:])
                        nc.vector.tensor_tensor(r[:], r[:], num[:], OP.mult)
                        nc.vector.tensor_scalar_mul(r[:], r[:], float(dd[h]))
                        nc.vector.tensor_tensor(wh2_cols[t][:], wh2_cols[t][:], r[:], OP.add)

                # gather Wh2
                w_in = dram.tile([R], F32, tag="ci_w2")
                w_out = dram.tile([N], F32, tag="co_w2")
                for t in range(RT):
                    nc.gpsimd.dma_start(
                        _col_ap(w_in[t * 128 : (t + 1) * 128], 128), wh2_cols[t][:]
                    )
                nc.gpsimd.collective_compute(
                    "AllGather", OP.bypass, replica_groups=rg,
                    ins=[w_in[:].opt()], outs=[w_out[:].opt()],
                )
                # ln(Wh2 + S) for the numerator (Wh2 can be negative)
                wsb = attp.tile([128, N], F32, tag="l2", bufs=2)
                nc.sync.dma_start(wsb[:], _bcast_ap(w_out[:], N))
                nc.vector.tensor_scalar_add(wsb[:], wsb[:], float(s_shift))
                lnw_b = pers.tile([128, N], F32, tag="lnx")  # reuse lnx slot
                nc.scalar.activation(lnw_b[:], wsb[:], AF.Ln)

                nd2 = att_pass(w_out, lnw_b, wh2_cols, float(a0), float(a1), "l2")
                for t, (num, den) in enumerate(nd2):
                    # true num = num_accum - S*den ; q = num/den
                    sden = cols.tile([128, 1], F32, tag=f"sd_{t}")
                    nc.vector.tensor_scalar_mul(sden[:], den[:], float(s_shift))
                    q = cols.tile([128, 1], F32, tag=f"q_{t}")
                    nc.vector.tensor_tensor(q[:], num[:], sden[:], OP.subtract)
                    r2 = cols.tile([128, 1], F32, tag=f"r2_{t}")
                    nc.vector.reciprocal(r2[:], den[:])
                    nc.vector.tensor_tensor(q[:], q[:], r2[:], OP.mult)
                    # elu(q) = max(q,0) + (exp(min(q,0)) - 1)
                    qn = cols.tile([128, 1], F32, tag=f"qn_{t}")
                    nc.vector.tensor_scalar_min(qn[:], q[:], 0.0)
                    nc.scalar.activation(qn[:], qn[:], AF.Exp)
                    nc.vector.tensor_scalar_add(qn[:], qn[:], -1.0)
                    nc.vector.tensor_scalar_max(q[:], q[:], 0.0)
                    nc.vector.tensor_tensor(q[:], q[:], qn[:], OP.add)
                    nc.gpsimd.dma_start(_col_ap(o_y[t * 128 : (t + 1) * 128], 128), q[:])

    return nc


def _ensure_ntff_hook():
    """Make trace=True work under axon: provide antenv.axon_hooks and
    register the ctypes NTFF hook from the boot helper (test-only path)."""
    import types, importlib
    try:
        from antenv.axon_hooks import get_axon_ntff_profile_hook  # noqa
        return
    except ImportError:
        pass
    import antenv
    mod = types.ModuleType("antenv.axon_hooks")
    mod._hook = None

    def set_axon_ntff_profile_hook(hook):
        mod._hook = hook

    def get_axon_ntff_profile_hook():
        return mod._hook

    mod.set_axon_ntff_profile_hook = set_axon_ntff_profile_hook
    mod.get_axon_ntff_profile_hook = get_axon_ntff_profile_hook
    sys.modules["antenv.axon_hooks"] = mod
    antenv.axon_hooks = mod
    try:
        if "/root/.axon_site" not in sys.path:
            sys.path.insert(0, "/root/.axon_site")
        from trn_agent_boot.trn_boot import _ntff_profile_via_ctypes
        hook = _ntff_profile_via_ctypes("/opt/axon/libaxon_pjrt.so")
        if hook is not None:
            set_axon_ntff_profile_hook(hook)
    except Exception as e:  # degrade: trace skipped, run still works
        print("ntff hook setup failed:", e)


def kernel(x, eps, adj, enc_w1, enc_b1, enc_w2, enc_b2, enc_w3, enc_b3,
           enc_w4, enc_b4, dec_w1, dec_b1, dec_w2, dec_b2, dec_w3, dec_b3,
           gat_W, gat_a, out_W, out_a, _trace=False):
    x = np.ascontiguousarray(np.asarray(x, np.float32))
    eps = np.ascontiguousarray(np.asarray(eps, np.float32))
    adj_u8 = np.ascontiguousarray(np.asarray(adj).astype(np.uint8))
    gat_W = np.asarray(gat_W, np.float32)
    gat_a = np.asarray(gat_a, np.float32)
    out_W = np.asarray(out_W, np.float32)
    out_a = np.asarray(out_a, np.float32)

    c1 = [float(gat_W[h, 0, :] @ gat_a[h, :64, 0]) for h in range(NH)]
    c2 = [float(gat_W[h, 0, :] @ gat_a[h, 64:, 0]) for h in range(NH)]
    dd = [float(gat_W[h, 0, :] @ out_W[h * 64:(h + 1) * 64, 0]) for h in range(NH)]
    a0 = float(out_a[0, 0])
    a1 = float(out_a[1, 0])
    # |Wh2| = |sum_h d_h t_h| < sum |d_h| since t_h is a convex combination
    # of x_hat values in (0,1); pad a little so ln(Wh2+S) stays finite.
    s_shift = float(sum(abs(d) for d in dd) + 1.0)

    w34 = np.concatenate([np.asarray(enc_w3), np.asarray(enc_w4)], 0)
    b34 = np.concatenate([np.asarray(enc_b3), np.asarray(enc_b4)], 0)

    nc = _build_program(c1, c2, dd, a0, a1, s_shift)

    def shard(arr, c, rows):
        return np.ascontiguousarray(np.asarray(arr, np.float32)[c * rows:(c + 1) * rows])

    in_maps = []
    for c in range(NC):
        in_maps.append({
            "w1": shard(enc_w1, c, 128), "b1": shard(enc_b1, c, 128),
            "w2": np.ascontiguousarray(np.asarray(enc_w2, np.float32)),
            "b2": np.ascontiguousarray(np.asarray(enc_b2, np.float32)),
            "w34": np.ascontiguousarray(w34.astype(np.float32)),
            "b34": np.ascontiguousarray(b34.astype(np.float32)),
            "dw1": np.ascontiguousarray(np.asarray(dec_w1, np.float32)),
            "db1": np.ascontiguousarray(np.asarray(dec_b1, np.float32)),
            "dw2": np.ascontiguousarray(np.asarray(dec_w2, np.float32)),
            "db2": np.ascontiguousarray(np.asarray(dec_b2, np.float32)),
            "dw3": shard(dec_w3, c, R), "db3": shard(dec_b3, c, R),
            "x": x, "eps": eps,
            "adj": np.ascontiguousarray(adj_u8[c * R:(c + 1) * R]),
        })

    if _trace is True:
        _ensure_ntff_hook()
    res = run_bass_kernel_spmd(nc, in_maps, core_ids=list(range(NC)),
                               trace=(_trace is True))

    xhat = np.concatenate([res.results[c]["o_xhat"] for c in range(NC)])
    mulv = res.results[0]["o_mulv"]
    y = np.concatenate([res.results[c]["o_y"] for c in range(NC)])
    out = (xhat.astype(np.float32),
           y.astype(np.float32)[:, None],
           mulv[:LAT].astype(np.float32),
           mulv[LAT:].astype(np.float32))
    if _trace:
        return out, res
    return out


# revision 22
# speedup vs baseline: 1.4155x; 1.2891x over previous
"""Trainium2 Bass kernel for nn_DeepIM (VAE + 2-layer rank-1 GAT on 4096 nodes).

Strategy (8 NeuronCores, SPMD):
  - VAE: mat-vec layers row-sharded 8 ways (memory-bound weight streaming
    split across cores); each layer output shard is AllGathered (~KB) and
    broadcast across partitions for the next layer.
  - GAT: Wh = x_hat[:,None] @ W is rank-1, so e_ij = c1*x_i + c2*x_j with
    host-precomputed scalars from the tiny W/a params. Each core owns
    N/8 = 512 query rows. Per (head, row-tile): the adjacency mask enters
    additively (adj ? 0 : -1e30) before Lrelu, ScalarE computes
    Lrelu -> Exp with a free row-sum (accum_out) giving the softmax
    denominator; the numerator uses exp(l + ln x_j) with one more fused
    Exp+accum. Softmax max-subtraction cancels in numer/denom, so skipped.
  - Layer 2 collapses identically: Wh2 = sum_h d_h * t_h (can be negative,
    so ln uses a +S shift, subtracted back out of the accumulated sum).

Compiler workarounds for this walrus build:
  - every instruction may carry at most ONE sync-wait: extra waits are
    split onto prefix NoOps on the same engine (see _split_waits_list).
  - tensor_tensor_reduce / raw-ISA DVE ops are broken ("ISA wrong length"):
    reductions use activation accum_out instead.
"""

import sys

if "/opt/trn_rl_repo" not in sys.path:
    sys.path.insert(0, "/opt/trn_rl_repo")

import numpy as np

import concourse.bass as bass
import concourse.mybir as mybir
import concourse.tile as tile
from concourse.vector_clock import ScopedClock
from concourse.bass_utils import run_bass_kernel_spmd

N = 4096
HID = 1024
LAT = 512
NH = 4
ALPHA = 0.2
NC = 8
R = N // NC          # 512 rows per core
RT = R // 128        # 4 row tiles per core
F32 = mybir.dt.float32
BF16 = mybir.dt.bfloat16
U8 = mybir.dt.uint8
NEG = -100.0

# ---------------------------------------------------------------------------
# Tile patches for this walrus build (1 sync-wait per instruction max).
# ---------------------------------------------------------------------------


def _split_waits_list(nc, insts):
    out = []
    for inst in insts:
        si = getattr(inst, "sync_info", None)
        if (
            si is not None
            and si.on_wait is not None
            and len(si.on_wait) > 1
            and getattr(inst, "engine", None) is not None
        ):
            waits = list(si.on_wait)
            for w in waits[:-1]:
                nop = mybir.InstNoOp(
                    name=nc.get_next_instruction_name(),
                    sync_info=mybir.SyncInfo(on_wait=[w], on_update=[]),
                    bass_nofuse=True,
                    engine=inst.engine,
                )
                out.append(nop)
            inst.sync_info = mybir.SyncInfo(
                on_wait=[waits[-1]], on_update=list(si.on_update or [])
            )
        out.append(inst)
    return out


_orig_lower = tile.TileContext._lower_ordered_insts


def _patched_lower(self, ordered):
    for bb in list(ordered.keys()):
        ordered[bb] = _split_waits_list(self.nc, ordered[bb])
    return _orig_lower(self, ordered)


def _drain_and_barrier_split(self, tick_clock, wait_clock):
    drain_inst = self.nc.sync.drain()
    wait_clock.add_sem_waits(
        drain_inst.ins, ScopedClock({None: tick_clock.global_clock})
    )
    si = drain_inst.ins.sync_info
    if si is not None and si.on_wait is not None and len(si.on_wait) > 1:
        waits = list(si.on_wait)
        drain_inst.ins.sync_info = mybir.SyncInfo(
            on_wait=waits[:1], on_update=list(si.on_update or [])
        )
        for i in range(1, len(waits)):
            d2 = self.nc.sync.drain()
            d2.ins.sync_info = mybir.SyncInfo(on_wait=[waits[i]], on_update=[])
    self.nc.all_engine_barrier()
    assert self.sems is not None
    popped = self.nc._tile_sem_poison_stack.pop()
    assert popped is self._sem_poison
    self.nc.clear_and_free_semaphores(list(self.sems.allocated().values()))
    self.nc.all_engine_barrier()


tile.TileContext._lower_ordered_insts = _patched_lower
tile.TileContext._drain_and_barrier = _drain_and_barrier_split


def _bcast_ap(ap, count, parts=128):
    """`count` contiguous elements at `ap`, replicated across partitions."""
    return bass.AP(tensor=ap.tensor, offset=ap.offset, ap=[[0, parts], [1, count]])


def _col_ap(ap, n):
    """[n] contiguous DRAM vector viewed as an [n, 1] column AP."""
    return bass.AP(tensor=ap.tensor, offset=ap.offset, ap=[[1, n], [1, 1]])


def _build_program(c1, c2, dd, a0, a1, s_shift):
    nc = bass.Bass(trn_type="TRN2", num_devices=NC)

    i_w1 = nc.dram_tensor("w1", [128, N], F32, kind="ExternalInput")
    i_b1 = nc.dram_tensor("b1", [128], F32, kind="ExternalInput")
    i_w2 = nc.dram_tensor("w2", [128, HID], F32, kind="ExternalInput")
    i_b2 = nc.dram_tensor("b2", [128], F32, kind="ExternalInput")
    i_w34 = nc.dram_tensor("w34", [128, HID], F32, kind="ExternalInput")
    i_b34 = nc.dram_tensor("b34", [128], F32, kind="ExternalInput")
    i_dw1 = nc.dram_tensor("dw1", [128, LAT], F32, kind="ExternalInput")
    i_db1 = nc.dram_tensor("db1", [128], F32, kind="ExternalInput")
    i_dw2 = nc.dram_tensor("dw2", [128, HID], F32, kind="ExternalInput")
    i_db2 = nc.dram_tensor("db2", [128], F32, kind="ExternalInput")
    i_dw3 = nc.dram_tensor("dw3", [R, HID], F32, kind="ExternalInput")
    i_db3 = nc.dram_tensor("db3", [R], F32, kind="ExternalInput")
    i_x = nc.dram_tensor("x", [N], F32, kind="ExternalInput")
    i_eps = nc.dram_tensor("eps", [LAT], F32, kind="ExternalInput")
    i_adj = nc.dram_tensor("adj", [R, N], U8, kind="ExternalInput")

    o_xhat = nc.dram_tensor("o_xhat", [R], F32, kind="ExternalOutput")
    o_mulv = nc.dram_tensor("o_mulv", [128], F32, kind="ExternalOutput")
    o_y = nc.dram_tensor("o_y", [R], F32, kind="ExternalOutput")

    AF = mybir.ActivationFunctionType
    OP = mybir.AluOpType
    rg = [list(range(NC))]

    with tile.TileContext(nc) as tc:
        with (
            tc.tile_pool(name="pers", bufs=1) as pers,
            tc.tile_pool(name="cols", bufs=1) as cols,
            tc.tile_pool(name="dram", bufs=1, space="DRAM") as dram,
        ):
            madj_t = []
            xh_cols = []
            xh_out = dram.tile([N], F32, tag="co_xh")
            with (
                tc.tile_pool(name="wpool", bufs=2) as wpool,
                tc.tile_pool(name="wpool1", bufs=1) as wpool1,
                tc.tile_pool(name="bc", bufs=1) as bc,
            ):
                # ---- additive adjacency mask: adj ? 0 : -1e30 (bf16) ----
                # (independent of the VAE chain; the scheduler runs it early)
                for t in range(RT):
                    a8 = wpool1.tile([128, N], U8, tag="a8")
                    nc.sync.dma_start(a8[:], i_adj[t * 128 : (t + 1) * 128, :])
                    af = wpool1.tile([128, N], F32, tag="af", bufs=4)
                    nc.vector.tensor_copy(af[:], a8[:])
                    mt = pers.tile([128, N], BF16, tag=f"madj{t}")
                    nc.vector.tensor_scalar(mt[:], af[:], -NEG, NEG, OP.mult, OP.add)
                    madj_t.append(mt)

                # ---------- VAE ----------
                def vae_layer(w_ap, b_ap, in_b, relu, tag, kcols, wtag="w", wp=None):
                    """mat-vec: h[p] = relu(sum_k w[p,k]*in_b[p,k] + b[p])."""
                    wt = (wp or wpool).tile([128, kcols], F32, tag=wtag)
                    nc.sync.dma_start(wt[:], w_ap)
                    bcol = cols.tile([128, 1], F32, tag=f"b_{tag}")
                    nc.gpsimd.dma_start(bcol[:], _col_ap(b_ap, 128))
                    q = wpool1.tile([128, kcols], F32, tag="af", bufs=4)
                    nc.vector.tensor_tensor(q[:], wt[:], in_b[:, :kcols], OP.mult)
                    hcol = cols.tile([128, 1], F32, tag=f"h_{tag}")
                    nc.scalar.activation(q[:], q[:], AF.Copy, accum_out=hcol[:])
                    nc.vector.tensor_tensor(hcol[:], hcol[:], bcol[:], OP.add)
                    if relu:
                        nc.vector.tensor_scalar_max(hcol[:], hcol[:], 0.0)
                    return hcol

                def ag_bcast(hcol, shard, tag):
                    full = shard * NC
                    cin = dram.tile([shard], F32, tag=f"ci_{tag}")
                    cout = dram.tile([full], F32, tag=f"co_{tag}")
                    nc.gpsimd.dma_start(_col_ap(cin[:], shard), hcol[:shard, :])
                    nc.gpsimd.collective_compute(
                        "AllGather", OP.bypass, replica_groups=rg,
                        ins=[cin[:].opt()], outs=[cout[:].opt()],
                    )
                    hb = bc.tile([128, full], F32, tag="hb")
                    nc.sync.dma_start(hb[:], _bcast_ap(cout[:], full))
                    return hb, cout

                def vae_layer_repl(w_h, b_h, in_b, relu, tag, kcols, nout):
                    """replicated mat-vec: every core computes all `nout`
                    outputs; result returned as a [128, nout] broadcast tile
                    via a local DRAM bounce (no collective)."""
                    nj = nout // 128
                    bounce = dram.tile([nout], F32, tag=f"bn_{tag}")
                    for j in range(nj):
                        hc = vae_layer(
                            w_h[j * 128 : (j + 1) * 128, :],
                            b_h[j * 128 : (j + 1) * 128],
                            in_b, relu, f"{tag}{j}", kcols,
                        )
                        nc.gpsimd.dma_start(
                            _col_ap(bounce[j * 128 : (j + 1) * 128], 128), hc[:]
                        )
                    hb = bc.tile([128, nout], F32, tag="hb")
                    nc.sync.dma_start(hb[:], _bcast_ap(bounce[:], nout))
                    return hb, bounce

                x_b = wpool1.tile([128, N], F32, tag="xbv")
                nc.sync.dma_start(x_b[:], _bcast_ap(i_x[:], N))

                h1 = vae_layer(i_w1[:, :], i_b1[:], x_b, True, "h1", N, wtag="w1", wp=wpool1)
                h1b, _ = ag_bcast(h1, 128, "h1")
                h2 = vae_layer(i_w2[:, :], i_b2[:], h1b, True, "h2", HID)
                h2b, _ = ag_bcast(h2, 128, "h2")
                ml = vae_layer(i_w34[:, :], i_b34[:], h2b, False, "ml", HID)
                nc.gpsimd.dma_start(_col_ap(o_mulv[:128], 128), ml[:])
                mlb, _ = ag_bcast(ml, 128, "ml")

                # z = mu + eps * exp(0.5*logvar)
                eps_b = bc.tile([128, LAT], F32, tag="epsb")
                nc.sync.dma_start(eps_b[:], _bcast_ap(i_eps[:], LAT))
                zb = bc.tile([128, LAT], F32, tag="zb")
                nc.scalar.activation(zb[:], mlb[:, LAT:], AF.Exp, scale=0.5)
                nc.vector.tensor_tensor(zb[:], zb[:], eps_b[:], OP.mult)
                nc.vector.tensor_tensor(zb[:], zb[:], mlb[:, :LAT], OP.add)

                h3 = vae_layer(i_dw1[:, :], i_db1[:], zb, True, "h3", LAT)
                h3b, _ = ag_bcast(h3, 128, "h3")
                h4 = vae_layer(i_dw2[:, :], i_db2[:], h3b, True, "h4", HID)
                h4b, _ = ag_bcast(h4, 128, "h4")

                for t in range(RT):
                    hc = vae_layer(
                        i_dw3[t * 128 : (t + 1) * 128, :],
                        i_db3[t * 128 : (t + 1) * 128],
                        h4b, False, f"d3_{t}", HID,
                    )
                    # sigmoid(v) = 1/(1+exp(-v))
                    sg = cols.tile([128, 1], F32, tag=f"sg_{t}")
                    nc.scalar.activation(sg[:], hc[:], AF.Exp, scale=-1.0)
                    nc.vector.tensor_scalar_add(sg[:], sg[:], 1.0)
                    nc.vector.reciprocal(sg[:], sg[:])
                    nc.gpsimd.dma_start(_col_ap(o_xhat[t * 128 : (t + 1) * 128], 128), sg[:])
                    xh_cols.append(sg)

                xh_in = dram.tile([R], F32, tag="ci_xh")
                for t in range(RT):
                    nc.gpsimd.dma_start(
                        _col_ap(xh_in[t * 128 : (t + 1) * 128], 128), xh_cols[t][:]
                    )
                nc.gpsimd.collective_compute(
                    "AllGather", OP.bypass, replica_groups=rg,
                    ins=[xh_in[:].opt()], outs=[xh_out[:].opt()],
                )

            # ---------- GAT ----------
            with tc.tile_pool(name="attp", bufs=1) as attp:
                # ln(x_hat) broadcast (numerator trick)
                stg = attp.tile([128, N], F32, tag="l2", bufs=2)
                nc.sync.dma_start(stg[:], _bcast_ap(xh_out[:], N))
                lnx_b = pers.tile([128, N], F32, tag="lnx")
                nc.scalar.activation(lnx_b[:], stg[:], AF.Ln)

                def att_pass(src_dram, lnvb, vcols, s1_scale, s2_scale, tag):
                    """per row tile: l = prelu(s2*v_j + s1*v_i + madj, 0.2);
                    den = sum_j exp(l); num = sum_j exp(l + lnv_j)."""
                    xcf = attp.tile([128, N], F32, tag="xcf", bufs=2)
                    nc.sync.dma_start(xcf[:], _bcast_ap(src_dram[:], N))
                    xc2 = attp.tile([128, N], BF16, tag="xc2", bufs=2)
                    nc.vector.tensor_scalar_mul(xc2[:], xcf[:], s2_scale)
                    outs = []
                    for t in range(RT):
                        s1c = cols.tile([128, 1], F32, tag=f"s1_{tag}_{t}")
                        nc.vector.tensor_scalar_mul(s1c[:], vcols[t][:], s1_scale)
                        tm = attp.tile([128, N], BF16, tag="tm", bufs=2)
                        nc.vector.tensor_tensor(tm[:], xc2[:], madj_t[t][:], OP.add)
                        lt = attp.tile([128, N], F32, tag="lt", bufs=1)
                        nc.scalar.activation(
                            lt[:], tm[:], AF.Prelu, bias=s1c[:], alpha=ALPHA
                        )
                        l2 = attp.tile([128, N], F32, tag="l2", bufs=2)
                        nc.vector.tensor_tensor(l2[:], lt[:], lnvb[:], OP.add)
                        den = cols.tile([128, 1], F32, tag=f"den_{tag}_{t}")
                        gs = attp.tile([128, N], BF16, tag="gs", bufs=1)
                        nc.scalar.activation(gs[:], lt[:], AF.Exp, accum_out=den[:])
                        num = cols.tile([128, 1], F32, tag=f"num_{tag}_{t}")
                        nc.scalar.activation(l2[:], l2[:], AF.Exp, accum_out=num[:])
                        outs.append((num, den))
                    return outs

                # layer 1: Wh2 = sum_h d_h * (num/den)
                wh2_cols = []
                for t in range(RT):
                    wc = cols.tile([128, 1], F32, tag=f"wh2_{t}")
                    nc.vector.memset(wc[:], 0.0)
                    wh2_cols.append(wc)
                for h in range(NH):
                    nd = att_pass(xh_out, lnx_b, xh_cols, c1[h], c2[h], f"h{h}")
                    for t, (num, den) in enumerate(nd):
                        r = cols.tile([128, 1], F32, tag=f"r_{t}")
                        nc.vector.reciprocal(r[:], den[:])
                        nc.vector.tensor_tensor(r[:], r[:], num[:], OP.mult)
                        nc.vector.tensor_scalar_mul(r[:], r[:], float(dd[h]))
                        nc.vector.tensor_tensor(wh2_cols[t][:], wh2_cols[t][:], r[:], OP.add)

                # gather Wh2
                w_in = dram.tile([R], F32, tag="ci_w2")
                w_out = dram.tile([N], F32, tag="co_w2")
                for t in range(RT):
                    nc.gpsimd.dma_start(
                        _col_ap(w_in[t * 128 : (t + 1) * 128], 128), wh2_cols[t][:]
                    )
                nc.gpsimd.collective_compute(
                    "AllGather", OP.bypass, replica_groups=rg,
                    ins=[w_in[:].opt()], outs=[w_out[:].opt()],
                )
                # ln(Wh2 + S) for the numerator (Wh2 can be negative)
                wsb = attp.tile([128, N], F32, tag="l2", bufs=2)
                nc.sync.dma_start(wsb[:], _bcast_ap(w_out[:], N))
                nc.vector.tensor_scalar_add(wsb[:], wsb[:], float(s_shift))
                lnw_b = pers.tile([128, N], F32, tag="lnx")  # reuse lnx slot
                nc.scalar.activation(lnw_b[:], wsb[:], AF.Ln)

                nd2 = att_pass(w_out, lnw_b, wh2_cols, float(a0), float(a1), "l2")
                for t, (num, den) in enumerate(nd2):
                    # true num = num_accum - S*den ; q = num/den
                    sden = cols.tile([128, 1], F32, tag=f"sd_{t}")
                    nc.vector.tensor_scalar_mul(sden[:], den[:], float(s_shift))
                    q = cols.tile([128, 1], F32, tag=f"q_{t}")
                    nc.vector.tensor_tensor(q[:], num[:], sden[:], OP.subtract)
                    r2 = cols.tile([128, 1], F32, tag=f"r2_{t}")
                    nc.vector.reciprocal(r2[:], den[:])
                    nc.vector.tensor_tensor(q[:], q[:], r2[:], OP.mult)
                    # elu(q) = max(q,0) + (exp(min(q,0)) - 1)
                    qn = cols.tile([128, 1], F32, tag=f"qn_{t}")
                    nc.vector.tensor_scalar_min(qn[:], q[:], 0.0)
                    nc.scalar.activation(qn[:], qn[:], AF.Exp)
                    nc.vector.tensor_scalar_add(qn[:], qn[:], -1.0)
                    nc.vector.tensor_scalar_max(q[:], q[:], 0.0)
                    nc.vector.tensor_tensor(q[:], q[:], qn[:], OP.add)
                    nc.gpsimd.dma_start(_col_ap(o_y[t * 128 : (t + 1) * 128], 128), q[:])

    return nc


def _ensure_ntff_hook():
    """Make trace=True work under axon: provide antenv.axon_hooks and
    register the ctypes NTFF hook from the boot helper (test-only path)."""
    import types, importlib
    try:
        from antenv.axon_hooks import get_axon_ntff_profile_hook  # noqa
        return
    except ImportError:
        pass
    import antenv
    mod = types.ModuleType("antenv.axon_hooks")
    mod._hook = None

    def set_axon_ntff_profile_hook(hook):
        mod._hook = hook

    def get_axon_ntff_profile_hook():
        return mod._hook

    mod.set_axon_ntff_profile_hook = set_axon_ntff_profile_hook
    mod.get_axon_ntff_profile_hook = get_axon_ntff_profile_hook
    sys.modules["antenv.axon_hooks"] = mod
    antenv.axon_hooks = mod
    try:
        if "/root/.axon_site" not in sys.path:
            sys.path.insert(0, "/root/.axon_site")
        from trn_agent_boot.trn_boot import _ntff_profile_via_ctypes
        hook = _ntff_profile_via_ctypes("/opt/axon/libaxon_pjrt.so")
        if hook is not None:
            set_axon_ntff_profile_hook(hook)
    except Exception as e:  # degrade: trace skipped, run still works
        print("ntff hook setup failed:", e)


def kernel(x, eps, adj, enc_w1, enc_b1, enc_w2, enc_b2, enc_w3, enc_b3,
           enc_w4, enc_b4, dec_w1, dec_b1, dec_w2, dec_b2, dec_w3, dec_b3,
           gat_W, gat_a, out_W, out_a, _trace=False):
    x = np.ascontiguousarray(np.asarray(x, np.float32))
    eps = np.ascontiguousarray(np.asarray(eps, np.float32))
    adj_u8 = np.ascontiguousarray(np.asarray(adj).astype(np.uint8))
    gat_W = np.asarray(gat_W, np.float32)
    gat_a = np.asarray(gat_a, np.float32)
    out_W = np.asarray(out_W, np.float32)
    out_a = np.asarray(out_a, np.float32)

    c1 = [float(gat_W[h, 0, :] @ gat_a[h, :64, 0]) for h in range(NH)]
    c2 = [float(gat_W[h, 0, :] @ gat_a[h, 64:, 0]) for h in range(NH)]
    dd = [float(gat_W[h, 0, :] @ out_W[h * 64:(h + 1) * 64, 0]) for h in range(NH)]
    a0 = float(out_a[0, 0])
    a1 = float(out_a[1, 0])
    # |Wh2| = |sum_h d_h t_h| < sum |d_h| since t_h is a convex combination
    # of x_hat values in (0,1); pad a little so ln(Wh2+S) stays finite.
    s_shift = float(sum(abs(d) for d in dd) + 1.0)

    w34 = np.concatenate([np.asarray(enc_w3), np.asarray(enc_w4)], 0)
    b34 = np.concatenate([np.asarray(enc_b3), np.asarray(enc_b4)], 0)

    nc = _build_program(c1, c2, dd, a0, a1, s_shift)

    def shard(arr, c, rows):
        return np.ascontiguousarray(np.asarray(arr, np.float32)[c * rows:(c + 1) * rows])

    in_maps = []
    for c in range(NC):
        in_maps.append({
            "w1": shard(enc_w1, c, 128), "b1": shard(enc_b1, c, 128),
            "w2": shard(enc_w2, c, 128), "b2": shard(enc_b2, c, 128),
            "w34": shard(w34, c, 128), "b34": shard(b34, c, 128),
            "dw1": shard(dec_w1, c, 128), "db1": shard(dec_b1, c, 128),
            "dw2": shard(dec_w2, c, 128), "db2": shard(dec_b2, c, 128),
            "dw3": shard(dec_w3, c, R), "db3": shard(dec_b3, c, R),
            "x": x, "eps": eps,
            "adj": np.ascontiguousarray(adj_u8[c * R:(c + 1) * R]),
        })

    if _trace is True:
        _ensure_ntff_hook()
    res = run_bass_kernel_spmd(nc, in_maps, core_ids=list(range(NC)),
                               trace=(_trace is True))

    xhat = np.concatenate([res.results[c]["o_xhat"] for c in range(NC)])
    mulv = np.concatenate([res.results[c]["o_mulv"] for c in range(NC)])
    y = np.concatenate([res.results[c]["o_y"] for c in range(NC)])
    out = (xhat.astype(np.float32),
           y.astype(np.float32)[:, None],
           mulv[:LAT].astype(np.float32),
           mulv[LAT:].astype(np.float32))
    if _trace:
        return out, res
    return out
